# revision 1
# baseline (speedup 1.0000x reference)
"""Trainium2 Bass kernel: 2-block Swin Transformer (W-MSA + SW-MSA) + patch merge.

Data-parallel over batch: 8 images -> 8 NeuronCores. Per core, everything runs
in channel-major layout (channels on SBUF partitions, tokens on the free axis).

Key design points:
  - LayerNorm gains/biases, qk-scale, and the mln gain are folded into the
    weights on the host; the SW-MSA mask is a precomputed {0,1} bf16 multiplier
    applied only to the 23 boundary windows.
  - LN stats via accumulating selector matmuls (per-chunk sums / sum-of-squares
    land on distinct partition rows of one PSUM tile), batched scalar math,
    then per-token rstd / m*rstd broadcast back through K=n_chunks selector
    matmuls (float32r runs at full PE speed for N>=256).
  - LN1 applies scatter straight into window-contiguous per-window-row tiles
    (including the SW-MSA cyclic roll), so qkv/attention reads are plain
    slices and x-hat is never materialized as a full tensor.
  - Attention: per-(window, head) S^T/O matmuls with operand slices at legal
    partition bases (matmul operands must share a base in {0,32,64}), ACT exp,
    block-wide softmax-denominator accumulation over P via sliding-selector
    matmuls, one reciprocal, selector-matmul broadcast, and normalization
    fused into an SBUF x PSUM multiply.
  - MLP streamed in 512-token chunks; bias+residual fused via
    scalar_tensor_tensor. Patch merge gathers 2x2 neighborhoods through
    strided APs and emits token-major matmuls, PSUM -> SBUF -> DRAM.
"""

import numpy as np
import ml_dtypes

import concourse.bacc as bacc
import concourse.mybir as mybir
import concourse.tile as tile
from concourse.bass_utils import run_bass_kernel_spmd

BF16 = mybir.dt.float16  # fp16: PE 1cy/row like bf16, 8x finer mantissa
F32 = mybir.dt.float32
F32R = mybir.dt.float32r
ALU = mybir.AluOpType
ACTF = mybir.ActivationFunctionType

P = 128
H = 96
WS = 8
NWI = 12
T = H * H
C = 128
NW = NWI * NWI
CH1 = 768
NCH1 = 12
CH2 = 512
NCH2 = 18
CM = 384
NCHM = 6
TMRG = 2304
EPS = 1e-5


def build_nc(debug=False):
    nc = bacc.Bacc(None, target_bir_lowering=False)

    x_d = nc.dram_tensor('x', (T, C), F32R, kind='ExternalInput')
    y_d = nc.dram_tensor('y', (TMRG, 2 * C), F32, kind='ExternalOutput')

    ident_d = nc.dram_tensor('ident', (P, P), F32R, kind='ExternalInput')
    onesE_d = nc.dram_tensor('onesE', (P, 128), F32R, kind='ExternalInput')
    onesCB_d = nc.dram_tensor('onesCB', (18, 18 * 128), F32R, kind='ExternalInput')
    onesel_d = nc.dram_tensor('onesel', (P, 96), BF16, kind='ExternalInput')
    selB_d = nc.dram_tensor('selB', (48, NWI * 128), F32R, kind='ExternalInput')
    onesrow_d = nc.dram_tensor('onesrow', (1, 128), BF16, kind='ExternalInput')
    bmask_d = nc.dram_tensor('bmask', (P, 64 * 23), BF16, kind='ExternalInput')
    wr_d = nc.dram_tensor('wr', (C, 4 * 256), BF16, kind='ExternalInput')
    br_d = nc.dram_tensor('br', (1, 256), BF16, kind='ExternalInput')

    blk_d = []
    for pre in ('a_', 'b_'):
        b = {}
        for nm, shp, dt in (
            ('wq', (C, C), BF16), ('wk', (C, C), BF16), ('wv', (C, C), BF16),
            ('bq', (C, 1), F32), ('bk', (C, 1), F32), ('bv', (1, C), BF16),
            ('wp', (C, C), BF16), ('bp', (C, 1), F32),
            ('w1', (C, 512), BF16), ('b1', (C, 4), F32),
            ('w2', (C, 512), BF16), ('b2', (C, 1), F32),
        ):
            b[nm] = nc.dram_tensor(pre + nm, shp, dt, kind='ExternalInput')
        blk_d.append(b)

    dbg = {}
    if debug:
        for nm in ('d_x0', 'd_xhA', 'd_r1', 'd_r2', 'd_xhB', 'd_r3', 'd_r4'):
            dbg[nm] = nc.dram_tensor(nm, (P, T), F32, kind='ExternalOutput')

    with tile.TileContext(nc) as tc:
        with (
            tc.tile_pool(name='cpool', bufs=1) as cpool,
            tc.tile_pool(name='xres_p', bufs=2) as xres_p,
            tc.tile_pool(name='otu_p', bufs=1) as otu_p,
            tc.tile_pool(name='xh_p', bufs=13) as xh_p,
            tc.tile_pool(name='sbm', bufs=2) as sbm,
            tc.tile_pool(name='sbq', bufs=1) as sbq,
            tc.tile_pool(name='scr_p', bufs=3) as scr_p,
            tc.tile_pool(name='sb2', bufs=2) as sb2,
            tc.tile_pool(name='hsb_p', bufs=3) as hsb_p,
            tc.tile_pool(name='sbst', bufs=3) as sbst,
            tc.tile_pool(name='sbsv', bufs=1) as sbsv,
            tc.tile_pool(name='stage_p', bufs=4) as stage_p,
            tc.tile_pool(name='psA', bufs=3, space='PSUM') as psA,
            tc.tile_pool(name='psC', bufs=1, space='PSUM') as psC,
        ):
            def ctile(dram, nm):
                t_ = cpool.tile(tuple(dram.shape), dram.dtype, name=nm)
                nc.sync.dma_start(t_[:], dram[:])
                return t_[:]

            ident = ctile(ident_d, 'ident')
            onesE = ctile(onesE_d, 'onesE')
            onesCB = ctile(onesCB_d, 'onesCB')
            onesel = ctile(onesel_d, 'onesel')
            selB = ctile(selB_d, 'selB')
            onesrow = ctile(onesrow_d, 'onesrow')
            bmask = ctile(bmask_d, 'bmask')
            wr = ctile(wr_d, 'wr')
            br = ctile(br_d, 'br')
            bw = []
            for i, b in enumerate(blk_d):
                bw.append({nm: ctile(b[nm], f'b{i}{nm}') for nm in b})

            # -------- input load + transpose to channel-major
            x0 = xres_p.tile((P, T), F32R, name='xres')
            for g in range(NCH2):
                tp = psA.tile((P, CH2), F32R, name='pA')
                for s in range(4):
                    st = stage_p.tile((P, P), F32R, name='stage')
                    nc.sync.dma_start(
                        st[:], x_d[512 * g + 128 * s:512 * g + 128 * (s + 1), :])
                    nc.tensor.transpose(tp[:, 128 * s:128 * (s + 1)], st[:], ident)
                nc.vector.tensor_copy(x0[:, 512 * g:512 * (g + 1)], tp[:])

            # -------- LN machinery
            def ln_stats(items, n_ch, chw, cdim):
                """items: (row, sum_rhs_ap, sq_rhs_ap|None).
                Returns a = rstd, b2 = m*rstd as (n_ch, chw) f32r tiles."""
                stats = psC.tile((64, chw), F32, name='pC')
                n_items = len(items)
                nsp = [(i_, min(512, chw - i_)) for i_ in range(0, chw, 512)]
                for ii, (row, s_ap, sq_ap) in enumerate(items):
                    for (n0, nn) in nsp:
                        sl = s_ap if chw <= 512 else s_ap[:, n0:n0 + nn]
                        # start must be set on the FIRST write of EACH psum
                        # region (bank), not just the first matmul overall
                        nc.tensor.matmul(stats[:, n0:n0 + nn],
                                         onesE[:, 63 - row:127 - row],
                                         sl,
                                         start=(ii == 0), stop=False,
                                         skip_group_check=True)
                    if sq_ap is None:
                        xsq = scr_p.tile((P, CH1), F32R, name='scr')
                        nc.gpsimd.tensor_tensor(xsq[:, :chw], s_ap, s_ap, ALU.mult)
                        sq_ap = xsq[:, :chw]
                    for (n0, nn) in nsp:
                        nc.tensor.matmul(stats[:, n0:n0 + nn],
                                         onesE[:, 31 - row:95 - row],
                                         sq_ap[:, n0:n0 + nn], start=False,
                                         stop=(ii == n_items - 1),
                                         skip_group_check=True)
                t1 = sbst.tile((n_ch, chw), F32, name='stt')
                nc.scalar.activation(t1[:], stats[0:n_ch, :], ACTF.Square)
                qc = sbst.tile((n_ch, chw), F32, name='stt')
                nc.scalar.activation(qc[:], stats[32:32 + n_ch, :], ACTF.Copy,
                                     scale=float(cdim))
                t2 = sbst.tile((n_ch, chw), F32, name='stt')
                nc.vector.tensor_tensor(t2[:], qc[:], t1[:], ALU.subtract)
                epst = sbst.tile((n_ch, 1), F32, name='epst')
                nc.gpsimd.memset(epst[:], EPS)
                sd = sbst.tile((n_ch, chw), F32, name='stt')
                # sd = sqrt(t2/cdim^2 + eps) = sigma
                nc.scalar.activation(sd[:], t2[:], ACTF.Sqrt, bias=epst[:],
                                     scale=1.0 / float(cdim * cdim))
                a_sb = sbsv.tile((n_ch, chw), F32R, name='a_sb')
                with nc.allow_low_precision(reason='f32r rstd is plenty for LN'):
                    nc.vector.reciprocal(a_sb[:], sd[:])
                b2_sb = sbsv.tile((n_ch, chw), F32R, name='b2_sb')
                # b2 = (sums * 1/cdim) * rstd = m * rstd
                nc.vector.scalar_tensor_tensor(b2_sb[:], stats[0:n_ch, :],
                                               1.0 / float(cdim), a_sb[:],
                                               ALU.mult, ALU.mult)
                return a_sb, b2_sb

            def ln_bcast(a_sb, b2_sb, ci, n_ch, chw):
                A = psA.tile((P, chw), F32, name='pA')
                B2 = psA.tile((P, chw), F32, name='pA')
                for n0 in range(0, chw, 512):
                    nn = min(512, chw - n0)
                    sel = onesCB[0:n_ch, 128 * ci:128 * ci + 128]
                    nc.tensor.matmul(A[:, n0:n0 + nn], sel, a_sb[:, n0:n0 + nn],
                                     skip_group_check=True)
                    nc.tensor.matmul(B2[:, n0:n0 + nn], sel, b2_sb[:, n0:n0 + nn],
                                     skip_group_check=True)
                return A, B2

            def win_view(ap_block):
                return ap_block.rearrange('p (wj r s) -> p r wj s', wj=NWI, r=WS, s=WS)

            def raster_view(ap_block):
                return ap_block.rearrange('p (r wj s) -> p r wj s', r=WS, wj=NWI, s=WS)

            # -------- LN1 -> streamed window-ordered x-hat tiles (one per wi)
            def ln1(src, rolled):
                xh = {}

                def get_xh(wi):
                    if wi not in xh:
                        xh[wi] = xh_p.tile((P, CH1), BF16, name='xh')
                    return xh[wi]

                chunks = [src[:, CH1 * ci:CH1 * (ci + 1)] for ci in range(NCH1)]
                a_sb, b2_sb = ln_stats(
                    [(ci, chunks[ci], None) for ci in range(NCH1)], NCH1, CH1, C)
                for ci in range(NCH1):
                    A, B2 = ln_bcast(a_sb, b2_sb, ci, NCH1, CH1)
                    u = scr_p.tile((P, CH1), F32R, name='scr')
                    nc.vector.tensor_tensor(u[:], chunks[ci], A[:], ALU.mult)
                    if not rolled:
                        nc.vector.tensor_tensor(
                            win_view(get_xh(ci)[:]),
                            raster_view(u[:]), raster_view(B2[:]), ALU.subtract)
                    else:
                        # two-step: contiguous subtract (2D PSUM read), then
                        # SBUF-only strided scatter on Pool (multi-dim strided
                        # PSUM reads crossing a bank boundary are broken)
                        xt = sb2.tile((P, CH1), BF16, name='xt')
                        nc.vector.tensor_tensor(xt[:], u[:], B2[:], ALU.subtract)
                        xv = xt[:].rearrange('p (r j) -> p r j', r=WS)
                        for (r0, wip, rw0) in ((0, (ci - 1) % NWI, 4), (4, ci, 0)):
                            ow = win_view(get_xh(wip)[:])
                            for (j0, wj0, nwj, s0, ns) in (
                                (4, 0, 11, 0, 8),
                                (92, 11, 1, 0, 4),
                                (0, 11, 1, 4, 4),
                            ):
                                jlen = (nwj - 1) * 8 + ns
                                sj = xv[:, r0:r0 + 4, j0:j0 + jlen]
                                if ns == 8:
                                    sj = sj.rearrange('p r (wj s) -> p r wj s', s=8)
                                else:
                                    sj = sj[:, :, None, :]
                                nc.gpsimd.tensor_copy(
                                    ow[:, rw0:rw0 + 4, wj0:wj0 + nwj, s0:s0 + ns],
                                    sj)
                return [xh[wi] for wi in range(NCH1)]

            # -------- attention
            def attention(blk, xh_tiles, r_in, r_out, rolled):
                den = psC.tile((48, CH1), F32, name='pC')
                otu = otu_p.tile((P, T), BF16, name='otu')
                n_splits = ((0, 512), (512, 256))
                for wi in range(NWI):
                    xhw = xh_tiles[wi][:]
                    q01 = psA.tile((64, CH1), F32, name='pA')
                    q23 = psA.tile((64, CH1), F32, name='pA')
                    k01 = psA.tile((64, CH1), F32, name='pA')
                    k23 = psA.tile((64, CH1), F32, name='pA')
                    v_ps = psA.tile((P, CH1), F32, name='pA')
                    for (n0, nn) in n_splits:
                        rhs = xhw[:, n0:n0 + nn]
                        nc.tensor.matmul(q01[:, n0:n0 + nn], blk['wq'][:, 0:64], rhs,
                                         skip_group_check=True)
                        nc.tensor.matmul(q23[:, n0:n0 + nn], blk['wq'][:, 64:128], rhs,
                                         skip_group_check=True)
                        nc.tensor.matmul(k01[:, n0:n0 + nn], blk['wk'][:, 0:64], rhs,
                                         skip_group_check=True)
                        nc.tensor.matmul(k23[:, n0:n0 + nn], blk['wk'][:, 64:128], rhs,
                                         skip_group_check=True)
                    for p_ in range(6):
                        nc.tensor.matmul(
                            v_ps[:, 128 * p_:128 * (p_ + 1)],
                            xhw[:, 128 * p_:128 * (p_ + 1)],
                            blk['wv'][:], start=True, stop=False,
                            skip_group_check=True)
                        nc.tensor.matmul(
                            v_ps[:, 128 * p_:128 * (p_ + 1)],
                            onesrow[:], blk['bv'][:], start=False, stop=True,
                            skip_group_check=True)
                    qa01 = sbq.tile((64, CH1), BF16, name='qa01')
                    nc.scalar.activation(qa01[:], q01[:], ACTF.Identity,
                                         bias=blk['bq'][0:64, :])
                    qa23 = sbq.tile((64, CH1), BF16, name='qa23')
                    nc.scalar.activation(qa23[:], q23[:], ACTF.Identity,
                                         bias=blk['bq'][64:128, :])
                    ka01 = sbq.tile((64, CH1), BF16, name='ka01')
                    nc.vector.tensor_scalar_add(ka01[:], k01[:], blk['bk'][0:64, :])
                    ka23 = sbq.tile((64, CH1), BF16, name='ka23')
                    nc.vector.tensor_scalar_add(ka23[:], k23[:], blk['bk'][64:128, :])
                    v_sb = sbm.tile((P, CH1), BF16, name='v_sb')
                    nc.vector.tensor_copy(v_sb[:], v_ps[:])
                    v_sw = sbm.tile((P, CH1), BF16, name='v_sw')
                    nc.scalar.activation(v_sw[0:64, :], v_ps[64:128, :], ACTF.Copy)
                    nc.scalar.activation(v_sw[64:128, :], v_ps[0:64, :], ACTF.Copy)
                    s01 = psA.tile((P, CH1), F32, name='pA')
                    s23 = psA.tile((P, CH1), F32, name='pA')
                    for w_ in range(NWI):
                        ws_ = slice(64 * w_, 64 * w_ + 64)
                        for (hh, k_sb, q_sb, s_ps) in (
                            (0, ka01, qa01, s01), (1, ka01, qa01, s01),
                            (2, ka23, qa23, s23), (3, ka23, qa23, s23),
                        ):
                            hrow = 32 * (hh % 2)
                            orow = 64 * (hh % 2)
                            nc.tensor.matmul(
                                s_ps[orow:orow + 64, ws_],
                                k_sb[hrow:hrow + 32, ws_],
                                q_sb[hrow:hrow + 32, ws_],
                                skip_group_check=True)
                    p01 = sbm.tile((P, CH1), BF16, name='p01')
                    nc.scalar.activation(p01[:], s01[:], ACTF.Exp)
                    p23 = sbm.tile((P, CH1), BF16, name='p23')
                    nc.scalar.activation(p23[:], s23[:], ACTF.Exp)
                    if rolled:
                        if wi == NWI - 1:
                            mslc = slice(0, CH1)
                            mk = bmask[:, 64 * 11:64 * 23]
                        else:
                            mslc = slice(64 * 11, CH1)
                            mk = bmask[:, 64 * wi:64 * wi + 64]
                        nc.vector.tensor_tensor(p01[:, mslc], p01[:, mslc], mk,
                                                ALU.mult)
                        nc.vector.tensor_tensor(p23[:, mslc], p23[:, mslc], mk,
                                                ALU.mult)
                    for ip, p_sb in ((0, p01), (1, p23)):
                        off = 48 - (4 * wi + 2 * ip)
                        for (n0, nn) in n_splits:
                            nc.tensor.matmul(
                                den[:, n0:n0 + nn], onesel[:, off:off + 48],
                                p_sb[:, n0:n0 + nn],
                                start=(wi == 0 and ip == 0),
                                stop=(wi == NWI - 1 and ip == 1),
                                skip_group_check=True)
                    o01 = psA.tile((64, CH1), F32, name='pA')
                    o23 = psA.tile((64, CH1), F32, name='pA')
                    for w_ in range(NWI):
                        ws_ = slice(64 * w_, 64 * w_ + 64)
                        vcol = 128 * (w_ // 2)
                        for hh in range(4):
                            p_sb = p01 if hh < 2 else p23
                            o_ps = o01 if hh < 2 else o23
                            b_ = 64 * (hh % 2)
                            vt = v_sb if (w_ % 2) == (hh % 2) else v_sw
                            nc.tensor.matmul(
                                o_ps[32 * (hh % 2):32 * (hh % 2) + 32, ws_],
                                vt[b_:b_ + 64, vcol + 32 * hh:vcol + 32 * hh + 32],
                                p_sb[b_:b_ + 64, ws_],
                                skip_group_check=True)
                    base = CH1 * wi
                    nc.scalar.activation(otu[0:64, base:base + CH1], o01[:], ACTF.Copy)
                    nc.scalar.activation(otu[64:128, base:base + CH1], o23[:],
                                         ACTF.Copy)
                rec = sbsv.tile((48, CH1), F32R, name='rec')
                with nc.allow_low_precision(reason='f32r softmax denom is plenty'):
                    nc.vector.reciprocal(rec[:], den[:])
                for wi in range(NWI):
                    base = CH1 * wi
                    R = psA.tile((P, CH1), F32, name='pA')
                    for (n0, nn) in n_splits:
                        nc.tensor.matmul(R[:, n0:n0 + nn],
                                         selB[:, 128 * wi:128 * wi + 128],
                                         rec[:, n0:n0 + nn], skip_group_check=True)
                    ot = sbm.tile((P, CH1), BF16, name='ot')
                    nc.vector.tensor_tensor(ot[:], otu[:, base:base + CH1], R[:],
                                            ALU.mult)
                    proj = psA.tile((P, CH1), F32, name='pA')
                    for (n0, nn) in n_splits:
                        nc.tensor.matmul(proj[:, n0:n0 + nn], blk['wp'][:],
                                         ot[:, n0:n0 + nn], skip_group_check=True)
                    pr = scr_p.tile((P, CH1), F32R, name='scr')
                    nc.scalar.activation(pr[:], proj[:], ACTF.Identity,
                                         bias=blk['bp'][:, :])
                    if not rolled:
                        nc.vector.tensor_tensor(
                            raster_view(r_out[:, base:base + CH1]),
                            win_view(pr[:]),
                            raster_view(r_in[:, base:base + CH1]),
                            ALU.add)
                    else:
                        pv = pr[:].rearrange('p (wj r s) -> p r wj s',
                                             wj=NWI, r=WS, s=WS)
                        riv = r_in.rearrange('p (i j) -> p i j', i=H)
                        rov = r_out.rearrange('p (i j) -> p i j', i=H)
                        for rp in range(2):
                            i0 = (8 * wi + 4 + 4 * rp) % H
                            for (wj0, nwj, s0, ns, c0) in (
                                (0, 11, 0, 8, 4),
                                (11, 1, 0, 4, 92),
                                (11, 1, 4, 4, 0),
                            ):
                                jlen = (nwj - 1) * 8 + ns
                                src = pv[:, 4 * rp:4 * rp + 4, wj0:wj0 + nwj,
                                         s0:s0 + ns]
                                din = riv[:, i0:i0 + 4, c0:c0 + jlen]
                                dout = rov[:, i0:i0 + 4, c0:c0 + jlen]
                                if ns == 8:
                                    din = din.rearrange('p r (wj s) -> p r wj s', s=8)
                                    dout = dout.rearrange('p r (wj s) -> p r wj s',
                                                          s=8)
                                else:
                                    din = din[:, :, None, :]
                                    dout = dout[:, :, None, :]
                                nc.vector.tensor_tensor(dout, src, din, ALU.add)

            # -------- MLP
            def mlp(blk, r_in, r_out):
                chunks = [r_in[:, CH2 * ci:CH2 * (ci + 1)] for ci in range(NCH2)]
                a_sb, b2_sb = ln_stats(
                    [(ci, chunks[ci], None) for ci in range(NCH2)], NCH2, CH2, C)
                for ci in range(NCH2):
                    A, B2 = ln_bcast(a_sb, b2_sb, ci, NCH2, CH2)
                    u = scr_p.tile((P, CH1), F32R, name='scr')
                    nc.vector.tensor_tensor(u[:, :CH2], chunks[ci], A[:], ALU.mult)
                    xh2 = sb2.tile((P, CH2), BF16, name='xh2')
                    nc.vector.tensor_tensor(xh2[:], u[:, :CH2], B2[:], ALU.subtract)
                    hsbs = []
                    for hq in range(4):
                        f_ps = psA.tile((P, CH2), F32, name='pA')
                        nc.tensor.matmul(f_ps[:], blk['w1'][:, 128 * hq:128 * hq + 128],
                                         xh2[:], skip_group_check=True)
                        hsb = hsb_p.tile((P, CH2), BF16, name='hsb')
                        nc.scalar.activation(hsb[:], f_ps[:], ACTF.Gelu,
                                             bias=blk['b1'][:, hq:hq + 1])
                        hsbs.append(hsb)
                    o_ps = psA.tile((P, CH2), F32, name='pA')
                    for g in range(4):
                        nc.tensor.matmul(o_ps[:], blk['w2'][:, 128 * g:128 * g + 128],
                                         hsbs[g][:], start=(g == 0), stop=(g == 3),
                                         skip_group_check=True)
                    nc.vector.scalar_tensor_tensor(
                        r_out[:, CH2 * ci:CH2 * (ci + 1)], o_ps[:], blk['b2'][:, :],
                        chunks[ci], ALU.add, ALU.add)

            # -------- patch merge
            def merge(r4):
                r4v = r4.rearrange('p (i j) -> p i j', i=H)
                groups = ((0, 0), (1, 0), (0, 1), (1, 1))

                def gap(g, ci):
                    di, dj = groups[g]
                    return r4v[:, di + 16 * ci:di + 16 * ci + 15:2, dj::2]

                items = []
                for ci in range(NCHM):
                    for g in range(4):
                        a = gap(g, ci)
                        xsq = scr_p.tile((P, CH1), F32R, name='scr')
                        nc.gpsimd.tensor_tensor(
                            xsq[:, :CM].rearrange('p (i j) -> p i j', j=48), a, a,
                            ALU.mult)
                        items.append((ci, a, xsq[:, :CM]))
                a_sb, b2_sb = ln_stats(items, NCHM, CM, 4 * C)
                for ci in range(NCHM):
                    A, B2 = ln_bcast(a_sb, b2_sb, ci, NCHM, CM)
                    Av = A[:].rearrange('p (i j) -> p i j', j=48)
                    Bv = B2[:].rearrange('p (i j) -> p i j', j=48)
                    xhc = sb2.tile((P, 4, CM), BF16, name='xhc')
                    for g in range(4):
                        u = scr_p.tile((P, CH1), F32R, name='scr')
                        uv = u[:, :CM].rearrange('p (i j) -> p i j', j=48)
                        nc.vector.tensor_tensor(uv, gap(g, ci), Av, ALU.mult)
                        nc.vector.tensor_tensor(
                            xhc[:, g, :].rearrange('p (i j) -> p i j', j=48),
                            uv, Bv, ALU.subtract)
                    for t3 in range(3):
                        yps = psA.tile((P, 256), F32, name='pA')
                        for g in range(4):
                            nc.tensor.matmul(
                                yps[:], xhc[:, g, 128 * t3:128 * t3 + 128],
                                wr[:, 256 * g:256 * g + 256],
                                start=(g == 0), stop=False, skip_group_check=True)
                        nc.tensor.matmul(yps[:], onesrow[:], br[:],
                                         start=False, stop=True,
                                         skip_group_check=True)
                        ysb = sb2.tile((P, 256), F32, name='ysb')
                        nc.scalar.activation(ysb[:], yps[:], ACTF.Copy)
                        nc.sync.dma_start(
                            y_d[CM * ci + 128 * t3:CM * ci + 128 * (t3 + 1), :],
                            ysb[:])

            def dump(nm, ap):
                if debug:
                    t_ = scr_p.tile((P, CH1), F32R, name='scr')
                    for c0 in range(0, ap.shape[1], CH1):
                        w_ = min(CH1, ap.shape[1] - c0)
                        nc.gpsimd.tensor_copy(t_[:, :w_], ap[:, c0:c0 + w_])
                        nc.sync.dma_start(dbg[nm][:, c0:c0 + w_],
                                          t_[:, :w_].bitcast(F32))

            def dump_tiles(nm, tiles):
                if debug:
                    for wi, tt_ in enumerate(tiles):
                        t_ = scr_p.tile((P, CH1), F32R, name='scr')
                        nc.vector.tensor_copy(t_[:], tt_[:])
                        nc.sync.dma_start(dbg[nm][:, CH1 * wi:CH1 * (wi + 1)],
                                          t_[:].bitcast(F32))

            # -------- pipeline
            xhA = ln1(x0[:], rolled=False)
            dump('d_x0', x0[:])
            dump_tiles('d_xhA', xhA)
            r1 = xres_p.tile((P, T), F32R, name='xres')
            attention(bw[0], xhA, x0[:], r1[:], rolled=False)
            dump('d_r1', r1[:])
            r2 = xres_p.tile((P, T), F32R, name='xres')
            mlp(bw[0], r1[:], r2[:])
            dump('d_r2', r2[:])
            xhB = ln1(r2[:], rolled=True)
            dump_tiles('d_xhB', xhB)
            r3 = xres_p.tile((P, T), F32R, name='xres')
            attention(bw[1], xhB, r2[:], r3[:], rolled=True)
            dump('d_r3', r3[:])
            r4 = xres_p.tile((P, T), F32R, name='xres')
            mlp(bw[1], r3[:], r4[:])
            dump('d_r4', r4[:])
            merge(r4[:])

    nc.compile()
    return nc


# ============================================================ host side

def _build_binmask():
    ws, ss = WS, WS // 2
    img = np.zeros((H, H), np.float32)
    cnt = 0
    for hs in (slice(0, -ws), slice(-ws, -ss), slice(-ss, None)):
        for wsl in (slice(0, -ws), slice(-ws, -ss), slice(-ss, None)):
            img[hs, wsl] = cnt
            cnt += 1
    mw = img.reshape(H // ws, ws, H // ws, ws).transpose(0, 2, 1, 3).reshape(-1, ws * ws)
    diff = mw[:, None, :] - mw[:, :, None]
    keep = (diff == 0).astype(np.float32)
    bm = np.zeros((P, 64 * 23), np.float32)
    slots = [(wi, 11) for wi in range(11)] + [(11, wj) for wj in range(NWI)]
    for sidx, (wi, wj) in enumerate(slots):
        w = NWI * wi + wj
        blk = keep[w].T
        bm[0:64, 64 * sidx:64 * sidx + 64] = blk
        bm[64:128, 64 * sidx:64 * sidx + 64] = blk
    return bm


def _host_consts():
    c = {}
    c['ident'] = np.eye(P, dtype=np.float32)
    oe = np.zeros((P, 128), np.float32)
    oe[:, 63] = 1.0
    c['onesE'] = oe
    m = np.zeros((18, 18 * 128), np.float32)
    for j in range(18):
        m[j, 128 * j:128 * j + 128] = 1.0
    c['onesCB'] = m
    os_ = np.zeros((P, 96), np.float32)
    os_[0:64, 48] = 1.0
    os_[64:128, 49] = 1.0
    c['onesel'] = os_
    sb = np.zeros((48, NWI * 128), np.float32)
    for wi in range(NWI):
        for mm_ in range(128):
            sb[4 * wi + mm_ // 32, 128 * wi + mm_] = 1.0
    c['selB'] = sb
    c['onesrow'] = np.ones((1, 128), np.float32)
    c['bmask'] = _build_binmask()
    return c


def _bf(x):
    return np.ascontiguousarray(np.asarray(x, np.float32)).astype(np.float16)


def _f32(x):
    return np.ascontiguousarray(np.asarray(x, np.float32))


def _fold_block(pre, inp):
    g1 = _f32(inp[pre + 'ln1_g']).astype(np.float64)
    b1 = _f32(inp[pre + 'ln1_b']).astype(np.float64)
    qkv_w = _f32(inp[pre + 'qkv_w']).astype(np.float64)
    qkv_b = _f32(inp[pre + 'qkv_b']).astype(np.float64)
    scale = (C // 4) ** -0.5
    w = g1[:, None] * qkv_w
    b = qkv_b + b1 @ qkv_w
    g2 = _f32(inp[pre + 'ln2_g']).astype(np.float64)
    bb2 = _f32(inp[pre + 'ln2_b']).astype(np.float64)
    fc1_w = _f32(inp[pre + 'fc1_w']).astype(np.float64)
    w1 = g2[:, None] * fc1_w
    b1f = _f32(inp[pre + 'fc1_b']).astype(np.float64) + bb2 @ fc1_w
    w2 = _f32(inp[pre + 'fc2_w'])
    return {
        'wq': _bf(w[:, 0:128] * scale),
        'wk': _bf(w[:, 128:256]),
        'wv': _bf(w[:, 256:384]),
        'bq': _f32(b[0:128] * scale).reshape(C, 1),
        'bk': _f32(b[128:256]).reshape(C, 1),
        'bv': _bf(b[256:384]).reshape(1, C),
        'wp': _bf(inp[pre + 'proj_w']),
        'bp': _f32(inp[pre + 'proj_b']).reshape(C, 1),
        'w1': _bf(w1),
        'b1': _f32(np.asarray(b1f).reshape(4, C).T),
        'w2': _bf(np.concatenate([w2[128 * g:128 * g + 128, :] for g in range(4)],
                                 axis=1)),
        'b2': _f32(inp[pre + 'fc2_b']).reshape(C, 1),
    }


def _fold_merge(inp):
    g = _f32(inp['mln_g']).astype(np.float64)
    b = _f32(inp['mln_b']).astype(np.float64)
    red = _f32(inp['red_w']).astype(np.float64)
    wrm = g[:, None] * red
    brm = b @ red
    wr_packed = np.concatenate([wrm[128 * g_:128 * g_ + 128, :] for g_ in range(4)],
                               axis=1)
    return _bf(wr_packed), _bf(brm).reshape(1, 256)


_NC_CACHE = {}


def _get_nc():
    if 'nc' not in _NC_CACHE:
        _NC_CACHE['nc'] = build_nc()
    return _NC_CACHE['nc']


def make_in_maps(inputs):
    consts = _host_consts()
    wrv, brv = _fold_merge(inputs)
    base = {
        'ident': consts['ident'], 'onesE': consts['onesE'],
        'onesCB': consts['onesCB'],
        'onesel': _bf(consts['onesel']), 'selB': consts['selB'],
        'onesrow': _bf(consts['onesrow']), 'bmask': _bf(consts['bmask']),
        'wr': wrv, 'br': brv,
    }
    for pre in ('a_', 'b_'):
        fold = _fold_block(pre, inputs)
        for nm, v in fold.items():
            base[pre + nm] = v
    x = _f32(inputs['x'])
    in_maps = []
    for b in range(x.shape[0]):
        m = dict(base)
        m['x'] = np.ascontiguousarray(x[b])
        in_maps.append(m)
    return in_maps


def kernel(**inputs):
    nc = _get_nc()
    in_maps = make_in_maps(inputs)
    res = run_bass_kernel_spmd(nc, in_maps, core_ids=list(range(len(in_maps))))
    out = np.stack([r['y'] for r in res.results], axis=0)
    return out.astype(np.float32)

